# revision 13
# baseline (speedup 1.0000x reference)
"""Causal self-attention Bass kernel for 8 trn2 NeuronCores.

Problem: B=4, T=2048, D=1024, H=16 causal self-attention (qkv proj + attn + out proj).

Sharding: core c = 2*b + g handles batch b (=c//2) and head-group g (=c%2, 8 heads).
Per core:
  - qkv projection column-shard: q,k,v columns for its 8 heads only.
  - flash-style attention in transposed-score layout sT[tk, tq]; softmax denominator
    via an extra ones-column in the AV matmul (row 64 of the [65, 512] psum output).
  - output projection row-shard (w_proj rows for its head dims) -> partial [T, D].
  - pairwise ReduceScatter {2b, 2b+1} sums the two head-group partials and splits
    output rows t: even core -> rows [0,1024), odd -> [1024, 2048).
Host reassembles by stacking the two halves per batch.

Precision: matmuls run as float32r (1 cyc/row for N>=256). Q/K path additionally
uses bf16 storage for xT / w_qk (softmax is shift-robust: score errors are absolute
and scores are O(1)). Value path (v, attn weights, projections) stays f32/f32r.
b_v is folded into beta = b_proj(once per pair) + w_proj_shard.T @ b_v_shard since
softmax rows sum to 1.
"""

from contextlib import ExitStack

import ml_dtypes
import numpy as np

import concourse.bass as bass
import concourse.mybir as mybir
import concourse.tile as tile
from concourse import bacc
from concourse.bass_utils import run_bass_kernel_spmd

B, T, D, H = 4, 2048, 1024, 16
HD = D // H  # 64
NCORES = 8
P = 128
f32 = mybir.dt.float32
f32r = mybir.dt.float32r
bf16 = mybir.dt.bfloat16
EXP = mybir.ActivationFunctionType.Exp

_CACHE = {}
LAST_RESULTS = None
_DEBUG_SINK = None


def _dbg(nc, name, ap):
    if _DEBUG_SINK is not None and name in _DEBUG_SINK:
        nc.sync.dma_start(_DEBUG_SINK[name].ap(), ap)


def _emit(nc, tc, x_d, wqk_d, wv_d, bqk_d, wproj_d, beta_d, out_d):
    with ExitStack() as ctx:
        # ---------------- constants / persistent tiles ----------------
        const = ctx.enter_context(tc.tile_pool(name="const", bufs=1))
        ident_f = const.tile([P, P], f32, tag="ident_f")
        nc.gpsimd.memset(ident_f[:], 0.0)
        nc.gpsimd.affine_select(
            out=ident_f[:], in_=ident_f[:],
            compare_op=mybir.AluOpType.not_equal, fill=1.0,
            base=0, pattern=[[-1, P]], channel_multiplier=1,
        )
        ident = const.tile([P, P], f32r, tag="ident")
        nc.vector.tensor_copy(ident[:], ident_f[:])
        # masks[i][p, f] = 1.0 if f - p - 128*i >= 0 else 0.0  (keep tq >= tk)
        masks = []
        for i in range(4):
            mk = const.tile([P, 512], f32, tag=f"mask{i}")
            nc.gpsimd.memset(mk[:], 1.0)
            nc.gpsimd.affine_select(
                out=mk[:], in_=mk[:],
                compare_op=mybir.AluOpType.is_ge, fill=0.0,
                base=-128 * i, pattern=[[1, 512]], channel_multiplier=-1,
            )
            masks.append(mk)
        bq = []
        for m in range(8):
            bt = const.tile([P, 1], f32, tag=f"bq{m}")
            nc.sync.dma_start(bt[:], bqk_d.ap()[m])
            bq.append(bt)
        beta_row = const.tile([1, D], f32, tag="beta_row")
        nc.sync.dma_start(beta_row[:], beta_d.ap())
        beta_b = const.tile([P, D], f32, tag="beta_b")
        nc.gpsimd.partition_broadcast(beta_b[:], beta_row[:], channels=P)
        _dbg(nc, "mask1", masks[1][:])
        _dbg(nc, "beta_b", beta_b[:])

        # persistent activations
        xt_pool = ctx.enter_context(tc.tile_pool(name="xt", bufs=1))
        xT = [xt_pool.tile([P, T], bf16, tag=f"xT{k}", name=f"xT{k}") for k in range(8)]
        vv_pool = ctx.enter_context(tc.tile_pool(name="vv", bufs=1))
        vv = [vv_pool.tile([P, 520], f32r, tag=f"vv{i}", name=f"vv{i}") for i in range(16)]
        on_pool = ctx.enter_context(tc.tile_pool(name="outn", bufs=1))
        outN = [on_pool.tile([P, T], f32r, tag=f"outN{mp}", name=f"outN{mp}") for mp in range(4)]
        ones8 = const.tile([P, 8], f32, tag="ones8")
        nc.vector.memset(ones8[:], 1.0)
        ones_src = ones8[:].rearrange("p (mp h one) -> p mp h one", mp=4, h=2)
        for i in range(16):
            dst = vv[i][:].rearrange("p (mp h d) -> p mp h d", mp=4, h=2)
            nc.vector.tensor_copy(dst[:, :, :, 64:65], ones_src[:, :, :, :])

        dram = ctx.enter_context(tc.tile_pool(name="dram", bufs=1, space="DRAM"))
        rs_in = dram.tile([T, D], f32)
        rs_out = dram.tile([T // 2, D], f32)

        # ---------------- phase 1: load x, transpose, compute v ----------------
        with ExitStack() as p1:
            xload = p1.enter_context(tc.tile_pool(name="xload", bufs=5))
            xtf = p1.enter_context(tc.tile_pool(name="xtf", bufs=1))
            wvp = p1.enter_context(tc.tile_pool(name="wv", bufs=3))
            tpps = p1.enter_context(tc.tile_pool(name="tpps", bufs=2, space="PSUM"))
            vps = p1.enter_context(tc.tile_pool(name="vps", bufs=2, space="PSUM"))
            xTf = [xtf.tile([P, 512], f32r, tag=f"xTf{k}", name=f"xTf{k}") for k in range(8)]
            for qq in range(4):  # t-quarters
                xi = []
                for ii in range(4):
                    xt_ = xload.tile([P, D], f32r, tag="x")
                    r0 = (qq * 4 + ii) * P
                    nc.sync.dma_start(xt_[:], x_d.ap()[r0 : r0 + P, :])
                    xi.append(xt_)
                for k in range(8):
                    tp = tpps.tile([P, 512], f32r, tag="tp")
                    for ii in range(4):
                        nc.tensor.transpose(
                            tp[:, ii * P : (ii + 1) * P],
                            xi[ii][:, k * P : (k + 1) * P],
                            ident[:],
                        )
                    # two evictions: f32 quarter copy (value path) + bf16 resident
                    nc.vector.tensor_copy(xTf[k][:], tp[:])
                    nc.vector.tensor_copy(xT[k][:, qq * 512 : (qq + 1) * 512], tp[:])
                # v for this quarter's 4 t-tiles
                for il in range(4):
                    i = qq * 4 + il
                    ps = vps.tile([P, 512], f32, tag="vp")
                    for k in range(8):
                        wv_t = wvp.tile([P, 512], f32r, tag="wvt")
                        nc.sync.dma_start(wv_t[:], wv_d.ap()[k * P : (k + 1) * P, :])
                        nc.tensor.matmul(
                            ps[:],
                            xTf[k][:, il * P : (il + 1) * P],
                            wv_t[:],
                            start=(k == 0), stop=(k == 7),
                        )
                    # strided evict: psum [p, (mp h d)] d=64 -> vv [p, (mp h d65)]
                    src = ps[:].rearrange("p (mp h d) -> p mp h d", mp=4, h=2)
                    dst = vv[i][:].rearrange("p (mp h d) -> p mp h d", mp=4, h=2)
                    nc.vector.tensor_copy(dst[:, :, :, 0:64], src[:, :, :, :])
            _dbg(nc, "xTf7", xTf[7][:])
            _dbg(nc, "xT0", xT[0][:])
            _dbg(nc, "vv0", vv[0][:])

        # ---------------- phase 2: per head-pair qkv + attention ----------------
        with ExitStack() as p2:
            qkt_pool = p2.enter_context(tc.tile_pool(name="qkt", bufs=1))
            qkT = [qkt_pool.tile([P, T], f32r, tag=f"qkT{m}", name=f"qkT{m}") for m in range(8)]
            wqkp = p2.enter_context(tc.tile_pool(name="wqk", bufs=8 if _DEBUG_SINK is not None else 16))
            atp = p2.enter_context(tc.tile_pool(name="atp", bufs=2 if _DEBUG_SINK is not None else 3))
            recip = p2.enter_context(tc.tile_pool(name="recip", bufs=2))
            bcast = p2.enter_context(tc.tile_pool(name="bcast", bufs=1))
            tmpb = p2.enter_context(tc.tile_pool(name="tmpb", bufs=1))
            qkps = p2.enter_context(tc.tile_pool(name="qkps", bufs=2, space="PSUM"))
            stps = p2.enter_context(tc.tile_pool(name="stps", bufs=2, space="PSUM"))
            oups = p2.enter_context(tc.tile_pool(name="oups", bufs=1, space="PSUM"))

            for mp in range(4):
                for m in (mp, 4 + mp):
                    for n in range(4):
                        ps = qkps.tile([P, 512], f32, tag="qkp")
                        for k in range(8):
                            wq_t = wqkp.tile([P, P], bf16, tag="wqkt")
                            nc.sync.dma_start(
                                wq_t[:],
                                wqk_d.ap()[k * P : (k + 1) * P, m * P : (m + 1) * P],
                            )
                            nc.tensor.matmul(
                                ps[:], wq_t[:],
                                xT[k][:, n * 512 : (n + 1) * 512],
                                start=(k == 0), stop=(k == 7),
                            )
                        nc.vector.tensor_scalar_add(
                            qkT[m][:, n * 512 : (n + 1) * 512], ps[:], bq[m][:]
                        )
                qs, ks = qkT[mp], qkT[4 + mp]
                for J in range(4):
                    nj = 4 * J + 4
                    ouA = oups.tile([65, 512], f32, tag="ouA")
                    ouB = oups.tile([65, 512], f32, tag="ouB")
                    Js = slice(J * 512, (J + 1) * 512)
                    for j in range(nj):
                        sT = stps.tile([P, 1024], f32, tag="sT")
                        js = slice(j * P, (j + 1) * P)
                        nc.tensor.matmul(
                            sT[:, 0:512],
                            ks[0:64, js], qs[0:64, Js],
                            start=True, stop=True, tile_position=(0, 0),
                        )
                        nc.tensor.matmul(
                            sT[:, 512:1024],
                            ks[64:128, js], qs[64:128, Js],
                            start=True, stop=True, tile_position=(64, 0),
                        )
                        at = atp.tile([P, 1024], f32r, tag="at")
                        nc.scalar.activation(at[:], sT[:], EXP, bias=0.0, scale=0.125)
                        i = j - 4 * J
                        if i >= 0:
                            nc.vector.tensor_mul(at[:, 0:512], at[:, 0:512], masks[i][:])
                            nc.vector.tensor_mul(at[:, 512:1024], at[:, 512:1024], masks[i][:])
                        if mp == 0 and J == 0 and j == 0:
                            _dbg(nc, "at000", at[:])
                        nc.tensor.matmul(
                            ouA[:], vv[j][:, 130 * mp : 130 * mp + 65],
                            at[:, 0:512],
                            start=(j == 0), stop=(j == nj - 1),
                        )
                        nc.tensor.matmul(
                            ouB[:], vv[j][:, 130 * mp + 65 : 130 * mp + 130],
                            at[:, 512:1024],
                            start=(j == 0), stop=(j == nj - 1),
                        )
                    # normalize by softmax denominator (psum row 64) and evict
                    if mp == 0 and J == 0 and _DEBUG_SINK is not None:
                        for _nm, _ou in (("ouA00", ouA), ("ouB00", ouB)):
                            if _nm in _DEBUG_SINK:
                                _dt = atp.tile([65, 512], f32, tag=f"dbg{_nm}", name=f"dbg{_nm}")
                                nc.vector.tensor_copy(_dt[:], _ou[:])
                                nc.sync.dma_start(_DEBUG_SINK[_nm].ap(), _dt[:])
                    rA = recip.tile([1, 512], f32, tag="rA")
                    rB = recip.tile([1, 512], f32, tag="rB")
                    nc.vector.reciprocal(rA[:], ouA[64:65, :])
                    nc.vector.reciprocal(rB[:], ouB[64:65, :])
                    bc = bcast.tile([64, 512], f32, tag="bc")
                    nc.gpsimd.partition_broadcast(bc[:, :], rA[:], channels=64)
                    bcB = bcast.tile([64, 512], f32, tag="bcB")
                    nc.gpsimd.partition_broadcast(bcB[:, :], rB[:], channels=64)
                    nc.vector.tensor_mul(outN[mp][0:64, Js], ouA[0:64, :], bc[:, :])
                    # head B must land on partitions 64-127: DVE cannot shift
                    # partitions, so normalize to a temp then DMA-shift.
                    tb = tmpb.tile([64, 512], f32r, tag="tb")
                    nc.vector.tensor_mul(tb[:], ouB[0:64, :], bcB[:, :])
                    nc.sync.dma_start(outN[mp][64:128, Js], tb[:])
            _dbg(nc, "qkT0", qkT[0][:])
            _dbg(nc, "qkT4", qkT[4][:])
            _dbg(nc, "outN0", outN[0][:])

        # ---------------- phase 3: output projection + ReduceScatter ----------------
        with ExitStack() as p3:
            wpp = p3.enter_context(tc.tile_pool(name="wpp", bufs=1))
            finp = p3.enter_context(tc.tile_pool(name="finp", bufs=3))
            fps = p3.enter_context(tc.tile_pool(name="fps", bufs=2, space="PSUM"))
            wproj_t = [wpp.tile([P, D], f32r, tag=f"wp{hp}", name=f"wp{hp}") for hp in range(4)]
            for hp in range(4):
                nc.sync.dma_start(wproj_t[hp][:], wproj_d.ap()[hp * P : (hp + 1) * P, :])
            for i in range(16):
                for n in range(2):
                    ps = fps.tile([P, 512], f32, tag="fp")
                    for hp in range(4):
                        nc.tensor.matmul(
                            ps[:],
                            outN[hp][:, i * P : (i + 1) * P],
                            wproj_t[hp][:, n * 512 : (n + 1) * 512],
                            start=(hp == 0), stop=(hp == 3),
                        )
                    fin = finp.tile([P, 512], f32, tag="fin")
                    nc.vector.tensor_add(fin[:], ps[:], beta_b[:, n * 512 : (n + 1) * 512])
                    nc.sync.dma_start(
                        rs_in[i * P : (i + 1) * P, n * 512 : (n + 1) * 512], fin[:]
                    )
            _dbg(nc, "rs_in", rs_in[:])
            nc.gpsimd.collective_compute(
                "ReduceScatter", mybir.AluOpType.add,
                replica_groups=[[0, 1], [2, 3], [4, 5], [6, 7]],
                ins=[rs_in.opt()], outs=[rs_out.opt()],
            )
            nc.sync.dma_start(out_d.ap(), rs_out[:])


def _build():
    if "nc" in _CACHE:
        return _CACHE["nc"]
    nc = bacc.Bacc("TRN2", target_bir_lowering=False, debug=False, num_devices=NCORES)
    x_d = nc.dram_tensor("x", [T, D], f32r, kind="ExternalInput")
    wqk_d = nc.dram_tensor("w_qk", [D, 1024], bf16, kind="ExternalInput")
    wv_d = nc.dram_tensor("w_v", [D, 512], f32r, kind="ExternalInput")
    bqk_d = nc.dram_tensor("b_qk", [8, P, 1], f32, kind="ExternalInput")
    wproj_d = nc.dram_tensor("w_proj", [512, D], f32r, kind="ExternalInput")
    beta_d = nc.dram_tensor("beta", [1, D], f32, kind="ExternalInput")
    out_d = nc.dram_tensor("out", [T // 2, D], f32, kind="ExternalOutput")
    with tile.TileContext(nc) as tc:
        _emit(nc, tc, x_d, wqk_d, wv_d, bqk_d, wproj_d, beta_d, out_d)
    nc.compile()
    _CACHE["nc"] = nc
    return nc


def make_in_maps(x, w_qkv, b_qkv, w_proj, b_proj):
    x = np.asarray(x, np.float32)
    w_qkv = np.asarray(w_qkv, np.float32)
    b_qkv = np.asarray(b_qkv, np.float32)
    w_proj = np.asarray(w_proj, np.float32)
    b_proj = np.asarray(b_proj, np.float32)
    in_maps = []
    for c in range(NCORES):
        b, g = c // 2, c % 2
        qcols = slice(g * 512, (g + 1) * 512)
        kcols = slice(D + g * 512, D + (g + 1) * 512)
        vcols = slice(2 * D + g * 512, 2 * D + (g + 1) * 512)
        w_qk = np.concatenate([w_qkv[:, qcols], w_qkv[:, kcols]], axis=1)
        b_qk = np.concatenate([b_qkv[qcols], b_qkv[kcols]])
        wp = np.ascontiguousarray(w_proj[g * 512 : (g + 1) * 512, :])
        beta = wp.T @ b_qkv[vcols]
        if g == 0:
            beta = beta + b_proj
        in_maps.append({
            "x": np.ascontiguousarray(x[b]),
            "w_qk": np.ascontiguousarray(w_qk).astype(ml_dtypes.bfloat16),
            "w_v": np.ascontiguousarray(w_qkv[:, vcols]),
            "b_qk": b_qk.reshape(8, P, 1),
            "w_proj": wp,
            "beta": beta.reshape(1, D).astype(np.float32),
        })
    return in_maps


def kernel(x, w_qkv, b_qkv, w_proj, b_proj, trace=False, **run_kwargs):
    global LAST_RESULTS
    nc = _build()
    in_maps = make_in_maps(x, w_qkv, b_qkv, w_proj, b_proj)
    res = run_bass_kernel_spmd(
        nc, in_maps, core_ids=list(range(NCORES)), trace=trace, **run_kwargs
    )
    LAST_RESULTS = res
    out = np.empty((B, T, D), np.float32)
    for b in range(B):
        out[b, : T // 2] = res.results[2 * b]["out"]
        out[b, T // 2 :] = res.results[2 * b + 1]["out"]
    return out


# revision 16
# speedup vs baseline: 1.8739x; 1.8739x over previous
"""Causal self-attention Bass kernel for 8 trn2 NeuronCores.

Problem: B=4, T=2048, D=1024, H=16 causal self-attention (qkv proj + attn + out proj).

Sharding: core c = 2*b + g handles batch b (=c//2) and head-group g (=c%2, 8 heads).
Per core:
  - qkv projection column-shard: q,k,v columns for its 8 heads only.
  - flash-style attention in transposed-score layout sT[tk, tq]; softmax denominator
    via an extra ones-column in the AV matmul (row 64 of the [65, 512] psum output).
  - output projection row-shard (w_proj rows for its head dims) -> partial [T, D].
  - pairwise ReduceScatter {2b, 2b+1} sums the two head-group partials and splits
    output rows t: even core -> rows [0,1024), odd -> [1024, 2048).
Host reassembles by stacking the two halves per batch.

Precision: matmuls run as float32r (1 cyc/row for N>=256). Q/K path additionally
uses bf16 storage for xT / w_qk (softmax is shift-robust: score errors are absolute
and scores are O(1)). Value path (v, attn weights, projections) stays f32/f32r.
b_v is folded into beta = b_proj(once per pair) + w_proj_shard.T @ b_v_shard since
softmax rows sum to 1.
"""

from contextlib import ExitStack

import ml_dtypes
import numpy as np

import concourse.bass as bass
import concourse.mybir as mybir
import concourse.tile as tile
from concourse import bacc
from concourse.bass_utils import run_bass_kernel_spmd

B, T, D, H = 4, 2048, 1024, 16
HD = D // H  # 64
NCORES = 8
P = 128
f32 = mybir.dt.float32
f32r = mybir.dt.float32r
bf16 = mybir.dt.bfloat16
EXP = mybir.ActivationFunctionType.Exp

_CACHE = {}
LAST_RESULTS = None
_DEBUG_SINK = None


def _dbg(nc, name, ap):
    if _DEBUG_SINK is not None and name in _DEBUG_SINK:
        nc.sync.dma_start(_DEBUG_SINK[name].ap(), ap)


def _emit(nc, tc, x_d, wqk_d, wv_d, bqk_d, wproj_d, beta_d, out_d):
    with ExitStack() as ctx:
        # ---------------- constants / persistent tiles ----------------
        const = ctx.enter_context(tc.tile_pool(name="const", bufs=1))
        ident_f = const.tile([P, P], f32, tag="ident_f")
        nc.gpsimd.memset(ident_f[:], 0.0)
        nc.gpsimd.affine_select(
            out=ident_f[:], in_=ident_f[:],
            compare_op=mybir.AluOpType.not_equal, fill=1.0,
            base=0, pattern=[[-1, P]], channel_multiplier=1,
        )
        ident = const.tile([P, P], f32r, tag="ident")
        nc.vector.tensor_copy(ident[:], ident_f[:])
        # triangle mask [128,128]: keep (1.0) iff f >= p
        mask_tri = const.tile([P, P], f32, tag="mask_tri")
        nc.gpsimd.memset(mask_tri[:], 1.0)
        nc.gpsimd.affine_select(
            out=mask_tri[:], in_=mask_tri[:],
            compare_op=mybir.AluOpType.is_ge, fill=0.0,
            base=0, pattern=[[1, P]], channel_multiplier=-1,
        )
        bq = []
        for m in range(8):
            bt = const.tile([P, 1], f32, tag=f"bq{m}")
            nc.sync.dma_start(bt[:], bqk_d.ap()[m])
            bq.append(bt)
        beta_row = const.tile([1, D], f32, tag="beta_row")
        nc.sync.dma_start(beta_row[:], beta_d.ap())
        beta_b = const.tile([P, D], f32, tag="beta_b")
        nc.gpsimd.partition_broadcast(beta_b[:], beta_row[:], channels=P)
        _dbg(nc, "beta_b", beta_b[:])

        # persistent activations
        xt_pool = ctx.enter_context(tc.tile_pool(name="xt", bufs=1))
        xT = [xt_pool.tile([P, T], bf16, tag=f"xT{k}", name=f"xT{k}") for k in range(8)]
        vv_pool = ctx.enter_context(tc.tile_pool(name="vv", bufs=1))
        vv = [vv_pool.tile([P, 520], f32r, tag=f"vv{i}", name=f"vv{i}") for i in range(16)]
        on_pool = ctx.enter_context(tc.tile_pool(name="outn", bufs=1))
        outN = [on_pool.tile([P, T], f32r, tag=f"outN{mp}", name=f"outN{mp}") for mp in range(4)]
        zeros384 = const.tile([P, 384], f32, tag="zeros384")
        nc.vector.memset(zeros384[:], 0.0)
        ones8 = const.tile([P, 8], f32, tag="ones8")
        nc.vector.memset(ones8[:], 1.0)
        ones_src = ones8[:].rearrange("p (mp h one) -> p mp h one", mp=4, h=2)
        for i in range(16):
            dst = vv[i][:].rearrange("p (mp h d) -> p mp h d", mp=4, h=2)
            nc.vector.tensor_copy(dst[:, :, :, 64:65], ones_src[:, :, :, :])

        dram = ctx.enter_context(tc.tile_pool(name="dram", bufs=1, space="DRAM"))
        rs_in = dram.tile([T, D], f32)
        rs_out = dram.tile([T // 2, D], f32)

        # ---------------- phase 1: load x, transpose, compute v ----------------
        with ExitStack() as p1:
            xload = p1.enter_context(tc.tile_pool(name="xload", bufs=5))
            xtf = p1.enter_context(tc.tile_pool(name="xtf", bufs=1))
            wvp = p1.enter_context(tc.tile_pool(name="wv", bufs=1))
            tpps = p1.enter_context(tc.tile_pool(name="tpps", bufs=2, space="PSUM"))
            vps = p1.enter_context(tc.tile_pool(name="vps", bufs=2, space="PSUM"))
            xTf = [xtf.tile([P, 512], f32r, tag=f"xTf{k}", name=f"xTf{k}") for k in range(8)]
            wv_t = []
            for k in range(8):
                wt = wvp.tile([P, 512], f32r, tag=f"wvt{k}", name=f"wvt{k}")
                nc.sync.dma_start(wt[:], wv_d.ap()[k * P : (k + 1) * P, :])
                wv_t.append(wt)
            for qq in range(4):  # t-quarters
                xi = []
                for ii in range(4):
                    xt_ = xload.tile([P, D], f32r, tag="x")
                    r0 = (qq * 4 + ii) * P
                    nc.sync.dma_start(xt_[:], x_d.ap()[r0 : r0 + P, :])
                    xi.append(xt_)
                for k in range(8):
                    tp = tpps.tile([P, 512], f32r, tag="tp")
                    for ii in range(4):
                        nc.tensor.transpose(
                            tp[:, ii * P : (ii + 1) * P],
                            xi[ii][:, k * P : (k + 1) * P],
                            ident[:],
                        )
                    # two evictions: f32 quarter copy (value path) + bf16 resident
                    nc.vector.tensor_copy(xTf[k][:], tp[:])
                    nc.vector.tensor_copy(xT[k][:, qq * 512 : (qq + 1) * 512], tp[:])
                # v for this quarter's 4 t-tiles
                for il in range(4):
                    i = qq * 4 + il
                    ps = vps.tile([P, 512], f32, tag="vp")
                    for k in range(8):
                        nc.tensor.matmul(
                            ps[:],
                            xTf[k][:, il * P : (il + 1) * P],
                            wv_t[k][:],
                            start=(k == 0), stop=(k == 7),
                        )
                    # strided evict: psum [p, (mp h d)] d=64 -> vv [p, (mp h d65)]
                    src = ps[:].rearrange("p (mp h d) -> p mp h d", mp=4, h=2)
                    dst = vv[i][:].rearrange("p (mp h d) -> p mp h d", mp=4, h=2)
                    nc.vector.tensor_copy(dst[:, :, :, 0:64], src[:, :, :, :])
            _dbg(nc, "xTf7", xTf[7][:])
            _dbg(nc, "xT0", xT[0][:])
            _dbg(nc, "vv0", vv[0][:])

        # ---------------- phase 2: per head-pair qkv + attention ----------------
        with ExitStack() as p2:
            qkt_pool = p2.enter_context(tc.tile_pool(name="qkt", bufs=1))
            qkT = [qkt_pool.tile([P, T], f32r, tag=f"qkT{m}", name=f"qkT{m}") for m in range(8)]
            wqkp = p2.enter_context(tc.tile_pool(name="wqk", bufs=2))
            atp = p2.enter_context(tc.tile_pool(name="atp", bufs=2 if _DEBUG_SINK is not None else 3))
            recip = p2.enter_context(tc.tile_pool(name="recip", bufs=2))
            bcast = p2.enter_context(tc.tile_pool(name="bcast", bufs=1))
            tmpb = p2.enter_context(tc.tile_pool(name="tmpb", bufs=1))
            qkps = p2.enter_context(tc.tile_pool(name="qkps", bufs=2, space="PSUM"))
            stps = p2.enter_context(tc.tile_pool(name="stps", bufs=2, space="PSUM"))
            oups = p2.enter_context(tc.tile_pool(name="oups", bufs=1, space="PSUM"))

            for mp in range(4):
                for m in (mp, 4 + mp):
                    wq_t = []
                    for k in range(8):
                        wt = wqkp.tile([P, P], bf16, tag=f"wqkt{k}", name=f"wqkt{k}")
                        nc.sync.dma_start(
                            wt[:],
                            wqk_d.ap()[k * P : (k + 1) * P, m * P : (m + 1) * P],
                        )
                        wq_t.append(wt)
                    for n in range(4):
                        ps = qkps.tile([P, 512], f32, tag="qkp")
                        for k in range(8):
                            nc.tensor.matmul(
                                ps[:], wq_t[k][:],
                                xT[k][:, n * 512 : (n + 1) * 512],
                                start=(k == 0), stop=(k == 7),
                            )
                        nc.vector.tensor_scalar_add(
                            qkT[m][:, n * 512 : (n + 1) * 512], ps[:], bq[m][:]
                        )
                qs, ks = qkT[mp], qkT[4 + mp]
                for J in range(4):
                    nj = 4 * J + 4
                    ouA = oups.tile([65, 512], f32, tag="ouA")
                    ouB = oups.tile([65, 512], f32, tag="ouB")
                    Js = slice(J * 512, (J + 1) * 512)
                    for j in range(nj):
                        sT = stps.tile([P, 1024], f32, tag="sT")
                        js = slice(j * P, (j + 1) * P)
                        nc.tensor.matmul(
                            sT[:, 0:512],
                            ks[0:64, js], qs[0:64, Js],
                            start=True, stop=True, tile_position=(0, 0),
                        )
                        nc.tensor.matmul(
                            sT[:, 512:1024],
                            ks[64:128, js], qs[64:128, Js],
                            start=True, stop=True, tile_position=(64, 0),
                        )
                        at = atp.tile([P, 1024], f32r, tag="at")
                        nc.scalar.activation(at[:], sT[:], EXP, bias=0.0, scale=0.125)
                        i = j - 4 * J
                        if i >= 0:
                            # diagonal-straddling block: zero cols < 128i, apply
                            # the triangle on cols [128i, 128i+128)
                            for h0 in (0, 512):
                                c0 = h0 + 128 * i
                                if i > 0:
                                    nc.vector.tensor_copy(
                                        at[:, h0 : h0 + 128 * i],
                                        zeros384[:, 0 : 128 * i],
                                    )
                                nc.vector.tensor_mul(
                                    at[:, c0 : c0 + 128],
                                    at[:, c0 : c0 + 128], mask_tri[:],
                                )
                        if mp == 0 and J == 0 and j == 0:
                            _dbg(nc, "at000", at[:])
                        nc.tensor.matmul(
                            ouA[:], vv[j][:, 130 * mp : 130 * mp + 65],
                            at[:, 0:512],
                            start=(j == 0), stop=(j == nj - 1),
                        )
                        nc.tensor.matmul(
                            ouB[:], vv[j][:, 130 * mp + 65 : 130 * mp + 130],
                            at[:, 512:1024],
                            start=(j == 0), stop=(j == nj - 1),
                        )
                    # normalize by softmax denominator (psum row 64) and evict
                    if mp == 0 and J == 0 and _DEBUG_SINK is not None:
                        for _nm, _ou in (("ouA00", ouA), ("ouB00", ouB)):
                            if _nm in _DEBUG_SINK:
                                _dt = atp.tile([65, 512], f32, tag=f"dbg{_nm}", name=f"dbg{_nm}")
                                nc.vector.tensor_copy(_dt[:], _ou[:])
                                nc.sync.dma_start(_DEBUG_SINK[_nm].ap(), _dt[:])
                    rA = recip.tile([1, 512], f32, tag="rA")
                    rB = recip.tile([1, 512], f32, tag="rB")
                    nc.vector.reciprocal(rA[:], ouA[64:65, :])
                    nc.vector.reciprocal(rB[:], ouB[64:65, :])
                    bc = bcast.tile([64, 512], f32, tag="bc")
                    nc.gpsimd.partition_broadcast(bc[:, :], rA[:], channels=64)
                    bcB = bcast.tile([64, 512], f32, tag="bcB")
                    nc.gpsimd.partition_broadcast(bcB[:, :], rB[:], channels=64)
                    nc.vector.tensor_mul(outN[mp][0:64, Js], ouA[0:64, :], bc[:, :])
                    # head B must land on partitions 64-127: DVE cannot shift
                    # partitions, so normalize to a temp then DMA-shift.
                    tb = tmpb.tile([64, 512], f32r, tag="tb")
                    nc.vector.tensor_mul(tb[:], ouB[0:64, :], bcB[:, :])
                    nc.sync.dma_start(outN[mp][64:128, Js], tb[:])
            _dbg(nc, "qkT0", qkT[0][:])
            _dbg(nc, "qkT4", qkT[4][:])
            _dbg(nc, "outN0", outN[0][:])

        # ---------------- phase 3: output projection + ReduceScatter ----------------
        with ExitStack() as p3:
            wpp = p3.enter_context(tc.tile_pool(name="wpp", bufs=1))
            finp = p3.enter_context(tc.tile_pool(name="finp", bufs=3))
            fps = p3.enter_context(tc.tile_pool(name="fps", bufs=2, space="PSUM"))
            wproj_t = [wpp.tile([P, D], f32r, tag=f"wp{hp}", name=f"wp{hp}") for hp in range(4)]
            for hp in range(4):
                nc.sync.dma_start(wproj_t[hp][:], wproj_d.ap()[hp * P : (hp + 1) * P, :])
            for i in range(16):
                for n in range(2):
                    ps = fps.tile([P, 512], f32, tag="fp")
                    for hp in range(4):
                        nc.tensor.matmul(
                            ps[:],
                            outN[hp][:, i * P : (i + 1) * P],
                            wproj_t[hp][:, n * 512 : (n + 1) * 512],
                            start=(hp == 0), stop=(hp == 3),
                        )
                    fin = finp.tile([P, 512], f32, tag="fin")
                    nc.vector.tensor_add(fin[:], ps[:], beta_b[:, n * 512 : (n + 1) * 512])
                    nc.sync.dma_start(
                        rs_in[i * P : (i + 1) * P, n * 512 : (n + 1) * 512], fin[:]
                    )
            _dbg(nc, "rs_in", rs_in[:])
            if globals().get("_NO_COLLECTIVE"):
                # profiling-only variant (TimelineSim is single-core)
                nc.sync.dma_start(out_d.ap(), rs_in[0 : T // 2, :])
            else:
                nc.gpsimd.collective_compute(
                    "ReduceScatter", mybir.AluOpType.add,
                    replica_groups=[[0, 1], [2, 3], [4, 5], [6, 7]],
                    ins=[rs_in.opt()], outs=[rs_out.opt()],
                )
                nc.sync.dma_start(out_d.ap(), rs_out[:])


def _build():
    if "nc" in _CACHE:
        return _CACHE["nc"]
    nc = bacc.Bacc("TRN2", target_bir_lowering=False, debug=False, num_devices=NCORES)
    x_d = nc.dram_tensor("x", [T, D], f32r, kind="ExternalInput")
    wqk_d = nc.dram_tensor("w_qk", [D, 1024], bf16, kind="ExternalInput")
    wv_d = nc.dram_tensor("w_v", [D, 512], f32r, kind="ExternalInput")
    bqk_d = nc.dram_tensor("b_qk", [8, P, 1], f32, kind="ExternalInput")
    wproj_d = nc.dram_tensor("w_proj", [512, D], f32r, kind="ExternalInput")
    beta_d = nc.dram_tensor("beta", [1, D], f32, kind="ExternalInput")
    out_d = nc.dram_tensor("out", [T // 2, D], f32, kind="ExternalOutput")
    with tile.TileContext(nc) as tc:
        _emit(nc, tc, x_d, wqk_d, wv_d, bqk_d, wproj_d, beta_d, out_d)
    nc.compile()
    _CACHE["nc"] = nc
    return nc


def make_in_maps(x, w_qkv, b_qkv, w_proj, b_proj):
    x = np.asarray(x, np.float32)
    w_qkv = np.asarray(w_qkv, np.float32)
    b_qkv = np.asarray(b_qkv, np.float32)
    w_proj = np.asarray(w_proj, np.float32)
    b_proj = np.asarray(b_proj, np.float32)
    in_maps = []
    for c in range(NCORES):
        b, g = c // 2, c % 2
        qcols = slice(g * 512, (g + 1) * 512)
        kcols = slice(D + g * 512, D + (g + 1) * 512)
        vcols = slice(2 * D + g * 512, 2 * D + (g + 1) * 512)
        w_qk = np.concatenate([w_qkv[:, qcols], w_qkv[:, kcols]], axis=1)
        b_qk = np.concatenate([b_qkv[qcols], b_qkv[kcols]])
        wp = np.ascontiguousarray(w_proj[g * 512 : (g + 1) * 512, :])
        beta = wp.T @ b_qkv[vcols]
        if g == 0:
            beta = beta + b_proj
        in_maps.append({
            "x": np.ascontiguousarray(x[b]),
            "w_qk": np.ascontiguousarray(w_qk).astype(ml_dtypes.bfloat16),
            "w_v": np.ascontiguousarray(w_qkv[:, vcols]),
            "b_qk": b_qk.reshape(8, P, 1),
            "w_proj": wp,
            "beta": beta.reshape(1, D).astype(np.float32),
        })
    return in_maps


def kernel(x, w_qkv, b_qkv, w_proj, b_proj, trace=False, **run_kwargs):
    global LAST_RESULTS
    nc = _build()
    in_maps = make_in_maps(x, w_qkv, b_qkv, w_proj, b_proj)
    res = run_bass_kernel_spmd(
        nc, in_maps, core_ids=list(range(NCORES)), trace=trace, **run_kwargs
    )
    LAST_RESULTS = res
    out = np.empty((B, T, D), np.float32)
    for b in range(B):
        out[b, : T // 2] = res.results[2 * b]["out"]
        out[b, T // 2 :] = res.results[2 * b + 1]["out"]
    return out
